# revision 1
# baseline (speedup 1.0000x reference)
"""Trainium2 Bass kernel for nn_AwareDecoder segment first/last gather.

Problem: input [16, 2048, 1024] f32, number_mask [16, 2048] int64 with ids in
[0, 512]. For each segment id i in [0, 512): find first/last row-major token
position with that id, gather those rows of the flattened input, concat ->
out [512, 2048] f32.

Strategy (8 NeuronCores, segment-sharded - no collectives):
  core c owns segments [64c, 64c+64). Each core:
    - DMAs the (tiny, 256KB) id array, extracts int64 low words,
    - computes per-segment min/max token position with an fp16 eq/select/
      reduce sweep on the vector engine. Token chunks sit on partitions and
      positions are encoded chunk-LOCALLY (values <= 256, fp16-exact) so the
      four mult/reduce passes run in the DVE 2x packed mode; the global
      position is reconstructed in the tiny post-transpose stage,
    - PE-transpose + free-axis reduce for the cross-partition combine,
    - gathers its 64 first + 64 last rows (4KB each) straight from HBM with
      one hardware indirect DMA (reads only 512KB of the 128MB input),
    - writes its [64, 2048] slice of the output.
Host concatenates the 8 slices.
"""
import numpy as np

import concourse.bass as bass
import concourse.tile as tile
from concourse import bacc, mybir
from concourse import bass_utils
from concourse.masks import make_identity

P = 128            # partitions
L = 32768          # B*S tokens
H = 1024           # hidden
NSEG = 512         # segments
NCORES = 8
SEG_PER_CORE = NSEG // NCORES            # 64
TOK_PER_PART = L // P                    # 256 tokens per partition
F32 = mybir.dt.float32
F16 = mybir.dt.float16
I32 = mybir.dt.int32


def build_nc():
    nc = bacc.Bacc("TRN2", target_bir_lowering=False, debug=False)

    x = nc.dram_tensor("x", [L, H], F32, kind="ExternalInput")
    # number_mask int64 raw bytes as int32 (lo, hi) pairs; partition p covers
    # tokens [p*256, (p+1)*256).
    idpairs = nc.dram_tensor("idpairs", [P, TOK_PER_PART, 2], I32, kind="ExternalInput")
    # packed fp16 consts (per-core): [c8hi (8*256) | c8lo (8*256) | posmin | posmax]
    cpack_in = nc.dram_tensor("cpack", [P, 18 * TOK_PER_PART], F16,
                              kind="ExternalInput")
    # global-position bases for the post-transpose decode:
    # rows 0..63   (min side): base[s, p] = (127 - p) * 256
    # rows 64..127 (max side): base[s, p] = p * 256
    base_in = nc.dram_tensor("base", [2, SEG_PER_CORE, P], F32, kind="ExternalInput")
    out = nc.dram_tensor("out", [SEG_PER_CORE, 2 * H], F32, kind="ExternalOutput")

    with tile.TileContext(nc) as tc:
        with tc.tile_pool(name="sb", bufs=1) as sb, \
             tc.tile_pool(name="big", bufs=1) as big, \
             tc.tile_pool(name="ps", bufs=1, space="PSUM") as ps:

            # ---- load ids, extract low int32 words, cast to fp16 ----
            idp_t = sb.tile([P, TOK_PER_PART, 2], I32)
            nc.sync.dma_start(idp_t[:], idpairs.ap())
            cpack = sb.tile([P, 18 * TOK_PER_PART], F16)
            nc.scalar.dma_start(cpack[:], cpack_in.ap())
            c8hi_t = cpack[:, 0:8 * TOK_PER_PART].rearrange(
                "p (a t) -> p a t", a=8)
            c8lo_t = cpack[:, 8 * TOK_PER_PART:16 * TOK_PER_PART].rearrange(
                "p (a t) -> p a t", a=8)
            posmin = cpack[:, 16 * TOK_PER_PART:17 * TOK_PER_PART]
            posmax = cpack[:, 17 * TOK_PER_PART:18 * TOK_PER_PART]
            base_t = sb.tile([P, P], F32)
            nc.gpsimd.dma_start(base_t[:], base_in.ap().rearrange("a s p -> (a s) p"))

            # ---- factorized seg compare: id>>3 == base/8 + m, id&7 == lo ----
            hi_i = sb.tile([P, TOK_PER_PART], I32)
            nc.vector.tensor_scalar(hi_i[:], idp_t[:, :, 0], 3, None,
                                    op0=mybir.AluOpType.arith_shift_right)
            lo_i = sb.tile([P, TOK_PER_PART], I32)
            nc.vector.tensor_scalar(lo_i[:], idp_t[:, :, 0], 7, None,
                                    op0=mybir.AluOpType.bitwise_and)
            hi_f = sb.tile([P, TOK_PER_PART], F16)
            nc.vector.tensor_copy(hi_f[:], hi_i[:])
            lo_f = sb.tile([P, TOK_PER_PART], F16)
            nc.vector.tensor_copy(lo_f[:], lo_i[:])

            eq_hi = sb.tile([P, 8, TOK_PER_PART], F16)
            nc.vector.tensor_tensor(
                out=eq_hi[:],
                in0=hi_f[:].unsqueeze(1).broadcast_to([P, 8, TOK_PER_PART]),
                in1=c8hi_t, op=mybir.AluOpType.is_equal)
            eq_lo = sb.tile([P, 8, TOK_PER_PART], F16)
            nc.vector.tensor_tensor(
                out=eq_lo[:],
                in0=lo_f[:].unsqueeze(1).broadcast_to([P, 8, TOK_PER_PART]),
                in1=c8lo_t, op=mybir.AluOpType.is_equal)
            eqlo_min = sb.tile([P, 8, TOK_PER_PART], F16)
            nc.vector.tensor_tensor(
                out=eqlo_min[:], in0=eq_lo[:],
                in1=posmin.unsqueeze(1).broadcast_to([P, 8, TOK_PER_PART]),
                op=mybir.AluOpType.mult)
            eqlo_max = sb.tile([P, 8, TOK_PER_PART], F16)
            nc.vector.tensor_tensor(
                out=eqlo_max[:], in0=eq_lo[:],
                in1=posmax.unsqueeze(1).broadcast_to([P, 8, TOK_PER_PART]),
                op=mybir.AluOpType.mult)

            # ---- big fused candidate passes (2x) + reduces ----
            cand = big.tile([P, 8, 8, TOK_PER_PART], F16)
            nc.vector.tensor_tensor(
                out=cand[:],
                in0=eq_hi[:].unsqueeze(2).broadcast_to([P, 8, 8, TOK_PER_PART]),
                in1=eqlo_min[:].unsqueeze(1).broadcast_to([P, 8, 8, TOK_PER_PART]),
                op=mybir.AluOpType.mult)
            # TT-max tree (2x) then small reduce: 256 -> 32 -> 1
            red = sb.tile([P, P], F16)  # [:, :64] min-enc, [:, 64:] max-enc
            c3 = cand[:].rearrange("p a b t -> p (a b) t")
            lv1 = big.tile([P, SEG_PER_CORE, 128], F16, tag="lv1")
            nc.vector.tensor_tensor(out=lv1[:], in0=c3[:, :, 0:128],
                                    in1=c3[:, :, 128:256], op=mybir.AluOpType.max)
            lv2 = sb.tile([P, SEG_PER_CORE, 64], F16, tag="lv2")
            nc.vector.tensor_tensor(out=lv2[:], in0=lv1[:, :, 0:64],
                                    in1=lv1[:, :, 64:128], op=mybir.AluOpType.max)
            lv3 = sb.tile([P, SEG_PER_CORE, 32], F16, tag="lv3")
            nc.vector.tensor_tensor(out=lv3[:], in0=lv2[:, :, 0:32],
                                    in1=lv2[:, :, 32:64], op=mybir.AluOpType.max)
            nc.vector.tensor_reduce(red[:, 0:SEG_PER_CORE], lv3[:],
                                    axis=mybir.AxisListType.X,
                                    op=mybir.AluOpType.max)
            cand2 = big.tile([P, 8, 8, TOK_PER_PART], F16)
            nc.vector.tensor_tensor(
                out=cand2[:],
                in0=eq_hi[:].unsqueeze(2).broadcast_to([P, 8, 8, TOK_PER_PART]),
                in1=eqlo_max[:].unsqueeze(1).broadcast_to([P, 8, 8, TOK_PER_PART]),
                op=mybir.AluOpType.mult)
            c3b = cand2[:].rearrange("p a b t -> p (a b) t")
            lv1b = big.tile([P, SEG_PER_CORE, 128], F16, tag="lv1")
            nc.vector.tensor_tensor(out=lv1b[:], in0=c3b[:, :, 0:128],
                                    in1=c3b[:, :, 128:256], op=mybir.AluOpType.max)
            lv2b = sb.tile([P, SEG_PER_CORE, 64], F16, tag="lv2")
            nc.vector.tensor_tensor(out=lv2b[:], in0=lv1b[:, :, 0:64],
                                    in1=lv1b[:, :, 64:128], op=mybir.AluOpType.max)
            lv3b = sb.tile([P, SEG_PER_CORE, 32], F16, tag="lv3")
            nc.vector.tensor_tensor(out=lv3b[:], in0=lv2b[:, :, 0:32],
                                    in1=lv2b[:, :, 32:64], op=mybir.AluOpType.max)
            nc.vector.tensor_reduce(red[:, SEG_PER_CORE:P], lv3b[:],
                                    axis=mybir.AxisListType.X,
                                    op=mybir.AluOpType.max)

            # ---- cross-partition combine, decode, gather ----
            ident = sb.tile([P, P], F16)
            make_identity(nc, ident[:])
            red_t = ps.tile([P, P], F16)
            nc.tensor.transpose(out=red_t[:], in_=red[:], identity=ident[:])
            mask = sb.tile([P, P], F32)
            nc.vector.tensor_scalar(mask[:], red_t[:], 0.0, None,
                                    op0=mybir.AluOpType.is_gt)
            glob = sb.tile([P, P], F32)
            nc.vector.tensor_tensor(out=glob[:], in0=red_t[:], in1=base_t[:],
                                    op=mybir.AluOpType.add)
            nc.vector.tensor_tensor(out=glob[:], in0=glob[:], in1=mask[:],
                                    op=mybir.AluOpType.mult)
            enc = sb.tile([P, 1], F32)
            nc.vector.tensor_reduce(enc[:], glob[:],
                                    axis=mybir.AxisListType.X,
                                    op=mybir.AluOpType.max)
            idx_f = sb.tile([P, 1], F32)
            nc.vector.tensor_scalar(idx_f[0:SEG_PER_CORE, :], enc[0:SEG_PER_CORE, :],
                                    -1.0, float(L),
                                    op0=mybir.AluOpType.mult,
                                    op1=mybir.AluOpType.add)
            nc.vector.tensor_scalar_add(idx_f[SEG_PER_CORE:P, :],
                                        enc[SEG_PER_CORE:P, :], -1.0)
            idx_i = sb.tile([P, 1], I32)
            nc.vector.tensor_copy(idx_i[:], idx_f[:])
            rows = big.tile([P, H], F32)
            nc.gpsimd.indirect_dma_start(
                out=rows[:], out_offset=None, in_=x.ap(),
                in_offset=bass.IndirectOffsetOnAxis(ap=idx_i[:, 0:1], axis=0))
            nc.gpsimd.dma_start(out.ap()[:, 0:H], rows[0:SEG_PER_CORE, :])
            nc.sync.dma_start(out.ap()[:, H:2 * H], rows[SEG_PER_CORE:P, :])

    nc.compile()
    return nc


_NC = None


def _get_nc():
    global _NC
    if _NC is None:
        _NC = build_nc()
    return _NC


def make_in_maps(input, number_mask):
    x = np.ascontiguousarray(np.asarray(input), dtype=np.float32).reshape(L, H)
    nm = np.ascontiguousarray(np.asarray(number_mask))
    if nm.dtype != np.int64:
        nm = nm.astype(np.int64)
    idpairs = nm.reshape(L).view(np.int32).reshape(P, TOK_PER_PART, 2)
    c8lo = np.repeat(np.arange(8, dtype=np.float16), TOK_PER_PART)
    f = np.arange(TOK_PER_PART, dtype=np.float16)
    pcol = np.arange(P, dtype=np.float32)
    base = np.empty((2, SEG_PER_CORE, P), dtype=np.float32)
    base[0] = (P - 1 - pcol) * TOK_PER_PART
    base[1] = pcol * TOK_PER_PART
    in_maps = []
    for c in range(NCORES):
        c8hi = np.repeat(np.arange(8, dtype=np.float16) + c * 8, TOK_PER_PART)
        cpack = np.tile(np.concatenate([c8hi, c8lo, TOK_PER_PART - f, f + 1]),
                        (P, 1))
        in_maps.append({"x": x, "idpairs": idpairs, "cpack": cpack,
                        "base": base})
    return in_maps


def kernel(input, number_mask, n, concat, **_):
    assert int(n) == NSEG and int(concat) == 1
    nc = _get_nc()
    in_maps = make_in_maps(input, number_mask)
    res = bass_utils.run_bass_kernel_spmd(nc, in_maps, core_ids=list(range(NCORES)))
    return np.concatenate([res.results[c]["out"] for c in range(NCORES)], axis=0)



# revision 3
# speedup vs baseline: 2.3580x; 2.3580x over previous
"""Trainium2 Bass kernel for nn_AwareDecoder segment first/last gather.

Problem: input [16, 2048, 1024] f32, number_mask [16, 2048] int64 with ids in
[0, 512]. For each segment id i in [0, 512): find first/last row-major token
position with that id, gather those rows of the flattened input, concat ->
out [512, 2048] f32.

Strategy (8 NeuronCores, segment-sharded - no collectives):
  core c owns segments [64c, 64c+64). Host passes ids narrowed to int16 and
  localized to the core's segment range (out-of-range -> -1). Each core:
    - DMAs the 64KB id tile across 4 HWDGE queues,
    - gpsimd local_scatter writes (global token pos + 1) into a per-partition
      [128, 64] segment table (ids within a 256-token partition row are
      unique, absent segments stay 0),
    - the min side is encoded as bitwise-NOT (65535 - x) masked by presence,
      so both first and last become max-reductions,
    - one PE transpose puts segments on partitions; two free-axis max
      reduces + affine decode yield the 128 gather indices,
    - one hardware indirect DMA gathers the 64 first + 64 last rows (512KB
      of the 128MB input), two direct DMAs write the [64, 2048] out slice.
Host concatenates the 8 slices.
"""
import numpy as np

import concourse.bass as bass
import concourse.tile as tile
from concourse import bacc, mybir
from concourse import bass_utils
from concourse.masks import make_identity

P = 128            # partitions
L = 32768          # B*S tokens
H = 1024           # hidden
NSEG = 512         # segments
NCORES = 8
SEG_PER_CORE = NSEG // NCORES            # 64
TOK_PER_PART = L // P                    # 256 tokens per partition
F32 = mybir.dt.float32
I32 = mybir.dt.int32
I16 = mybir.dt.int16
U16 = mybir.dt.uint16


def build_nc():
    nc = bacc.Bacc("TRN2", target_bir_lowering=False, debug=False)

    x = nc.dram_tensor("x", [L, H], F32, kind="ExternalInput")
    # per-core localized ids: value in [0, 64) for own segments, -1 otherwise
    ids_in = nc.dram_tensor("ids16", [P, TOK_PER_PART], I16, kind="ExternalInput")
    out = nc.dram_tensor("out", [SEG_PER_CORE, 2 * H], F32, kind="ExternalOutput")

    with tile.TileContext(nc) as tc:
        with tc.tile_pool(name="sb", bufs=1) as sb, \
             tc.tile_pool(name="ps", bufs=1, space="PSUM") as ps:

            # ---- constants generated on-chip while the id DMA is in flight
            data = sb.tile([P, TOK_PER_PART], U16)
            nc.gpsimd.iota(data[:], pattern=[[1, TOK_PER_PART]], base=1,
                           channel_multiplier=TOK_PER_PART)
            ident = sb.tile([P, P], F32)
            make_identity(nc, ident[:])

            # ---- id tile in via the 2 HWDGE queues
            ids_t = sb.tile([P, TOK_PER_PART], I16)
            nc.sync.dma_start(ids_t[0:64, :], ids_in.ap()[0:64, :])
            nc.scalar.dma_start(ids_t[64:128, :], ids_in.ap()[64:128, :])

            # ---- scatter: table[p, s] = global pos + 1 of s's occurrence in
            # partition p (0 if absent). Lands in the right half of M.
            M = sb.tile([P, P], U16)
            nc.gpsimd.local_scatter(
                out_ap=M[:, SEG_PER_CORE:P], data_ap=data[:], idxs_ap=ids_t[:],
                channels=P, num_elems=SEG_PER_CORE, num_idxs=TOK_PER_PART)

            # ---- min-side encoding in the left half: (65535 - table) * (table > 0)
            rev = sb.tile([P, SEG_PER_CORE], U16)
            nc.vector.tensor_scalar(rev[:], M[:, SEG_PER_CORE:P], 65535, None,
                                    op0=mybir.AluOpType.bitwise_xor)
            msk = sb.tile([P, SEG_PER_CORE], U16)
            nc.vector.tensor_scalar(msk[:], M[:, SEG_PER_CORE:P], 0, None,
                                    op0=mybir.AluOpType.is_gt)
            nc.vector.tensor_tensor(out=M[:, 0:SEG_PER_CORE], in0=rev[:],
                                    in1=msk[:], op=mybir.AluOpType.mult)

            # ---- transpose so segments sit on partitions
            Mf = sb.tile([P, P], F32)
            nc.vector.tensor_copy(Mf[:], M[:])
            T = ps.tile([P, P], F32)
            nc.tensor.transpose(out=T[:], in_=Mf[:], identity=ident[:])

            # rows 0..63: max of reversed encoding -> first; rows 64..127:
            # max of direct encoding -> last.
            enc = sb.tile([P, 1], F32)
            nc.vector.tensor_reduce(enc[0:SEG_PER_CORE, :], T[0:SEG_PER_CORE, :],
                                    axis=mybir.AxisListType.X,
                                    op=mybir.AluOpType.max)
            nc.vector.tensor_reduce(enc[SEG_PER_CORE:P, :], T[SEG_PER_CORE:P, :],
                                    axis=mybir.AxisListType.X,
                                    op=mybir.AluOpType.max)
            # decode: first = 65534 - maxrev, last = enc - 1
            idx_f = sb.tile([P, 1], F32)
            nc.vector.tensor_scalar(idx_f[0:SEG_PER_CORE, :],
                                    enc[0:SEG_PER_CORE, :], -1.0, 65534.0,
                                    op0=mybir.AluOpType.mult,
                                    op1=mybir.AluOpType.add)
            nc.vector.tensor_scalar_add(idx_f[SEG_PER_CORE:P, :],
                                        enc[SEG_PER_CORE:P, :], -1.0)
            idx_i = sb.tile([P, 1], I32)
            nc.vector.tensor_copy(idx_i[:], idx_f[:])

            # ---- gather the 128 rows, write the out slice on 2 queues
            rows = sb.tile([P, H], F32)
            nc.gpsimd.indirect_dma_start(
                out=rows[:], out_offset=None, in_=x.ap(),
                in_offset=bass.IndirectOffsetOnAxis(ap=idx_i[:, 0:1], axis=0))
            nc.sync.dma_start(out.ap()[:, 0:H], rows[0:SEG_PER_CORE, :])
            nc.scalar.dma_start(out.ap()[:, H:2 * H], rows[SEG_PER_CORE:P, :])

    nc.compile()
    return nc


_NC = None


def _get_nc():
    global _NC
    if _NC is None:
        _NC = build_nc()
    return _NC


def make_in_maps(input, number_mask):
    x = np.ascontiguousarray(np.asarray(input), dtype=np.float32).reshape(L, H)
    nm = np.asarray(number_mask).reshape(L).astype(np.int16)
    in_maps = []
    for c in range(NCORES):
        loc = (nm - SEG_PER_CORE * c).astype(np.int16)
        loc[(loc < 0) | (loc >= SEG_PER_CORE)] = -1
        in_maps.append({"x": x, "ids16": loc.reshape(P, TOK_PER_PART)})
    return in_maps


def kernel(input, number_mask, n, concat, **_):
    assert int(n) == NSEG and int(concat) == 1
    nc = _get_nc()
    in_maps = make_in_maps(input, number_mask)
    res = bass_utils.run_bass_kernel_spmd(nc, in_maps, core_ids=list(range(NCORES)))
    return np.concatenate([res.results[c]["out"] for c in range(NCORES)], axis=0)


# revision 8
# speedup vs baseline: 2.6531x; 1.1251x over previous
"""Trainium2 Bass kernel for nn_AwareDecoder segment first/last gather.

Problem: input [16, 2048, 1024] f32, number_mask [16, 2048] int64 with ids in
[0, 512]. For each segment id i in [0, 512): find first/last row-major token
position with that id, gather those rows of the flattened input, concat ->
out [512, 2048] f32.

Strategy (8 NeuronCores, segment-sharded - no collectives):
  core c owns segments [64c, 64c+64). Host passes ids narrowed to int16 and
  localized to the core's segment range (out-of-range -> -1). Each core:
    - DMAs the 64KB id tile across 4 HWDGE queues,
    - gpsimd local_scatter writes (global token pos + 1) into a per-partition
      [128, 64] segment table (ids within a 256-token partition row are
      unique, absent segments stay 0),
    - the min side is encoded as bitwise-NOT (65535 - x) masked by presence,
      so both first and last become max-reductions,
    - one PE transpose puts segments on partitions; two free-axis max
      reduces + affine decode yield the 128 gather indices,
    - one hardware indirect DMA gathers the 64 first + 64 last rows (512KB
      of the 128MB input), two direct DMAs write the [64, 2048] out slice.
Host concatenates the 8 slices.
"""
import numpy as np

import concourse.bass as bass
import concourse.tile as tile
from concourse import bacc, mybir
from concourse import bass_utils
from concourse.masks import make_identity

P = 128            # partitions
L = 32768          # B*S tokens
H = 1024           # hidden
NSEG = 512         # segments
NCORES = 8
SEG_PER_CORE = NSEG // NCORES            # 64
TOK_PER_PART = L // P                    # 256 tokens per partition
F32 = mybir.dt.float32
I32 = mybir.dt.int32
I16 = mybir.dt.int16
U16 = mybir.dt.uint16


def build_nc():
    nc = bacc.Bacc("TRN2", target_bir_lowering=False, debug=False)

    x = nc.dram_tensor("x", [L, H], F32, kind="ExternalInput")
    # per-core localized ids: value in [0, 64) for own segments, -1 otherwise
    ids_in = nc.dram_tensor("ids16", [P, TOK_PER_PART], I16, kind="ExternalInput")
    out = nc.dram_tensor("out", [SEG_PER_CORE, 2 * H], F32, kind="ExternalOutput")

    with tile.TileContext(nc) as tc:
        with tc.tile_pool(name="sb", bufs=1) as sb, \
             tc.tile_pool(name="ps", bufs=1, space="PSUM") as ps:

            # ---- constants generated on-chip while the id DMA is in flight
            data = sb.tile([P, TOK_PER_PART], U16)
            nc.gpsimd.iota(data[:], pattern=[[1, TOK_PER_PART]], base=1,
                           channel_multiplier=TOK_PER_PART)
            ident = sb.tile([P, P], F32)
            make_identity(nc, ident[:])

            # ---- id tile in via the 2 HWDGE queues
            ids_t = sb.tile([P, TOK_PER_PART], I16)
            nc.sync.dma_start(ids_t[0:64, :], ids_in.ap()[0:64, :])
            nc.scalar.dma_start(ids_t[64:128, :], ids_in.ap()[64:128, :])

            # ---- scatter: table[p, s] = global pos + 1 of s's occurrence in
            # partition p (0 if absent). Lands in the right half of M.
            M = sb.tile([P, P], U16)
            nc.gpsimd.local_scatter(
                out_ap=M[:, SEG_PER_CORE:P], data_ap=data[:], idxs_ap=ids_t[:],
                channels=P, num_elems=SEG_PER_CORE, num_idxs=TOK_PER_PART)

            # ---- min-side encoding in the left half: (65535 - table) * (table > 0)
            rev = sb.tile([P, SEG_PER_CORE], U16)
            nc.vector.tensor_scalar(rev[:], M[:, SEG_PER_CORE:P], 65535, None,
                                    op0=mybir.AluOpType.bitwise_xor)
            msk = sb.tile([P, SEG_PER_CORE], U16)
            nc.vector.tensor_scalar(msk[:], M[:, SEG_PER_CORE:P], 0, None,
                                    op0=mybir.AluOpType.is_gt)
            nc.vector.tensor_tensor(out=M[:, 0:SEG_PER_CORE], in0=rev[:],
                                    in1=msk[:], op=mybir.AluOpType.mult)

            # ---- transpose so segments sit on partitions
            Mf = sb.tile([P, P], F32)
            nc.vector.tensor_copy(Mf[:], M[:])
            T = ps.tile([P, P], F32)
            nc.tensor.transpose(out=T[:], in_=Mf[:], identity=ident[:])

            # rows 0..63: max of reversed encoding -> first; rows 64..127:
            # max of direct encoding -> last.
            enc = sb.tile([P, 1], F32)
            nc.vector.tensor_reduce(enc[0:SEG_PER_CORE, :], T[0:SEG_PER_CORE, :],
                                    axis=mybir.AxisListType.X,
                                    op=mybir.AluOpType.max)
            nc.vector.tensor_reduce(enc[SEG_PER_CORE:P, :], T[SEG_PER_CORE:P, :],
                                    axis=mybir.AxisListType.X,
                                    op=mybir.AluOpType.max)
            # decode: first = 65534 - maxrev, last = enc - 1
            idx_f = sb.tile([P, 1], F32)
            nc.vector.tensor_scalar(idx_f[0:SEG_PER_CORE, :],
                                    enc[0:SEG_PER_CORE, :], -1.0, 65534.0,
                                    op0=mybir.AluOpType.mult,
                                    op1=mybir.AluOpType.add)
            nc.vector.tensor_scalar_add(idx_f[SEG_PER_CORE:P, :],
                                        enc[SEG_PER_CORE:P, :], -1.0)
            idx_i = sb.tile([P, 1], I32)
            nc.vector.tensor_copy(idx_i[:], idx_f[:])

            # ---- gather the 128 rows, write the out slice on 2 queues
            rows = sb.tile([P, H], F32)
            nc.gpsimd.indirect_dma_start(
                out=rows[:], out_offset=None, in_=x.ap(),
                in_offset=bass.IndirectOffsetOnAxis(ap=idx_i[:, 0:1], axis=0))
            nc.sync.dma_start(out.ap()[:, 0:H], rows[0:SEG_PER_CORE, :])
            nc.scalar.dma_start(out.ap()[:, H:2 * H], rows[SEG_PER_CORE:P, :])

    nc.compile()
    return nc


_NC = None


def _get_nc():
    global _NC
    if _NC is None:
        _NC = build_nc()
    return _NC


def make_in_maps(input, number_mask):
    x = np.ascontiguousarray(np.asarray(input), dtype=np.float32).reshape(L, H)
    nm = np.asarray(number_mask).reshape(L).astype(np.int16)
    in_maps = []
    for c in range(NCORES):
        loc = (nm - SEG_PER_CORE * c).astype(np.int16)
        loc[(loc < 0) | (loc >= SEG_PER_CORE)] = -1
        in_maps.append({"x": x, "ids16": loc.reshape(P, TOK_PER_PART)})
    return in_maps


def kernel(input, number_mask, n, concat, **_):
    assert int(n) == NSEG and int(concat) == 1
    nc = _get_nc()
    in_maps = make_in_maps(input, number_mask)
    res = bass_utils.run_bass_kernel_spmd(nc, in_maps, core_ids=list(range(NCORES)))
    return np.concatenate([res.results[c]["out"] for c in range(NCORES)], axis=0)


# revision 11
# speedup vs baseline: 2.7560x; 1.0388x over previous
"""Trainium2 Bass kernel for nn_AwareDecoder segment first/last gather.

Problem: input [16, 2048, 1024] f32, number_mask [16, 2048] int64 with ids in
[0, 512]. For each segment id i in [0, 512): find first/last row-major token
position with that id, gather those rows of the flattened input, concat ->
out [512, 2048] f32.

Strategy (8 NeuronCores, segment-sharded - no collectives):
  core c owns segments [64c, 64c+64). Host passes ids narrowed to int16 and
  localized to the core's segment range (out-of-range -> -1). Each core:
    - DMAs the 64KB id tile across 4 HWDGE queues,
    - gpsimd local_scatter writes (global token pos + 1) into a per-partition
      [128, 64] segment table (ids within a 256-token partition row are
      unique, absent segments stay 0),
    - the min side is encoded as bitwise-NOT (65535 - x) masked by presence,
      so both first and last become max-reductions,
    - one PE transpose puts segments on partitions; two free-axis max
      reduces + affine decode yield the 128 gather indices,
    - one hardware indirect DMA gathers the 64 first + 64 last rows (512KB
      of the 128MB input), two direct DMAs write the [64, 2048] out slice.
Host concatenates the 8 slices.
"""
import numpy as np

import concourse.bass as bass
import concourse.tile as tile
from concourse import bacc, mybir
from concourse import bass_utils
from concourse.masks import make_identity

P = 128            # partitions
L = 32768          # B*S tokens
H = 1024           # hidden
NSEG = 512         # segments
NCORES = 8
SEG_PER_CORE = NSEG // NCORES            # 64
TOK_PER_PART = L // P                    # 256 tokens per partition
F32 = mybir.dt.float32
I32 = mybir.dt.int32
I16 = mybir.dt.int16
U16 = mybir.dt.uint16


def build_nc():
    nc = bacc.Bacc("TRN2", target_bir_lowering=False, debug=False)

    x = nc.dram_tensor("x", [L, H], F32, kind="ExternalInput")
    # per-core localized ids: value in [0, 64) for own segments, -1 otherwise
    ids_in = nc.dram_tensor("ids16", [P, TOK_PER_PART], I16, kind="ExternalInput")
    out = nc.dram_tensor("out", [SEG_PER_CORE, 2 * H], F32, kind="ExternalOutput")

    with tile.TileContext(nc) as tc:
        with tc.tile_pool(name="sb", bufs=1) as sb, \
             tc.tile_pool(name="ps", bufs=1, space="PSUM") as ps:

            # ---- constants generated on-chip while the id DMA is in flight
            data = sb.tile([P, TOK_PER_PART], U16)
            nc.gpsimd.iota(data[:], pattern=[[1, TOK_PER_PART]], base=1,
                           channel_multiplier=TOK_PER_PART)
            ident = sb.tile([P, P], F32)
            make_identity(nc, ident[:])

            # ---- id tile in via the 2 HWDGE queues
            ids_t = sb.tile([P, TOK_PER_PART], I16)
            nc.sync.dma_start(ids_t[0:64, :], ids_in.ap()[0:64, :])
            nc.scalar.dma_start(ids_t[64:128, :], ids_in.ap()[64:128, :])

            # ---- scatter: table[p, s] = global pos + 1 of s's occurrence in
            # partition p (0 if absent). Lands in the right half of M.
            M = sb.tile([P, P], U16)
            nc.gpsimd.local_scatter(
                out_ap=M[:, SEG_PER_CORE:P], data_ap=data[:], idxs_ap=ids_t[:],
                channels=P, num_elems=SEG_PER_CORE, num_idxs=TOK_PER_PART)

            # ---- min-side encoding in the left half: (65535 - table) * (table > 0)
            rev = sb.tile([P, SEG_PER_CORE], U16)
            nc.vector.tensor_scalar(rev[:], M[:, SEG_PER_CORE:P], 65535, None,
                                    op0=mybir.AluOpType.bitwise_xor)
            msk = sb.tile([P, SEG_PER_CORE], U16)
            nc.vector.tensor_scalar(msk[:], M[:, SEG_PER_CORE:P], 0, None,
                                    op0=mybir.AluOpType.is_gt)
            nc.vector.tensor_tensor(out=M[:, 0:SEG_PER_CORE], in0=rev[:],
                                    in1=msk[:], op=mybir.AluOpType.mult)

            # ---- two transposes so both index halves land on partitions 0:64
            Mf = sb.tile([P, P], F32)
            nc.vector.tensor_copy(Mf[:], M[:])
            T1 = ps.tile([SEG_PER_CORE, P], F32, tag="t1")
            nc.tensor.transpose(out=T1[:], in_=Mf[:, 0:SEG_PER_CORE],
                                identity=ident[:])
            T2 = ps.tile([SEG_PER_CORE, P], F32, tag="t2")
            nc.tensor.transpose(out=T2[:], in_=Mf[:, SEG_PER_CORE:P],
                                identity=ident[:])

            encA = sb.tile([SEG_PER_CORE, 1], F32)
            nc.vector.tensor_reduce(encA[:], T1[:], axis=mybir.AxisListType.X,
                                    op=mybir.AluOpType.max)
            encB = sb.tile([SEG_PER_CORE, 1], F32)
            nc.vector.tensor_reduce(encB[:], T2[:], axis=mybir.AxisListType.X,
                                    op=mybir.AluOpType.max)
            # decode: first = 65534 - maxrev, last = enc - 1
            idxA_f = sb.tile([SEG_PER_CORE, 1], F32)
            nc.vector.tensor_scalar(idxA_f[:], encA[:], -1.0, 65534.0,
                                    op0=mybir.AluOpType.mult,
                                    op1=mybir.AluOpType.add)
            idxB_f = sb.tile([SEG_PER_CORE, 1], F32)
            nc.vector.tensor_scalar_add(idxB_f[:], encB[:], -1.0)
            idxA = sb.tile([SEG_PER_CORE, 1], I32)
            nc.vector.tensor_copy(idxA[:], idxA_f[:])
            idxB = sb.tile([SEG_PER_CORE, 1], I32)
            nc.vector.tensor_copy(idxB[:], idxB_f[:])

            # ---- gather the rows with two indirect DMAs, write on 2 queues
            rowsA = sb.tile([SEG_PER_CORE, H], F32)
            rowsB = sb.tile([SEG_PER_CORE, H], F32)
            nc.gpsimd.indirect_dma_start(
                out=rowsA[:], out_offset=None, in_=x.ap(),
                in_offset=bass.IndirectOffsetOnAxis(ap=idxA[:, 0:1], axis=0))
            nc.gpsimd.indirect_dma_start(
                out=rowsB[:], out_offset=None, in_=x.ap(),
                in_offset=bass.IndirectOffsetOnAxis(ap=idxB[:, 0:1], axis=0))
            nc.sync.dma_start(out.ap()[:, 0:H], rowsA[:])
            nc.scalar.dma_start(out.ap()[:, H:2 * H], rowsB[:])

    nc.compile()
    return nc


_NC = None


def _get_nc():
    global _NC
    if _NC is None:
        _NC = build_nc()
    return _NC


def make_in_maps(input, number_mask):
    x = np.ascontiguousarray(np.asarray(input), dtype=np.float32).reshape(L, H)
    nm = np.asarray(number_mask).reshape(L).astype(np.int16)
    in_maps = []
    for c in range(NCORES):
        loc = (nm - SEG_PER_CORE * c).astype(np.int16)
        loc[(loc < 0) | (loc >= SEG_PER_CORE)] = -1
        in_maps.append({"x": x, "ids16": loc.reshape(P, TOK_PER_PART)})
    return in_maps


def kernel(input, number_mask, n, concat, **_):
    assert int(n) == NSEG and int(concat) == 1
    nc = _get_nc()
    in_maps = make_in_maps(input, number_mask)
    res = bass_utils.run_bass_kernel_spmd(nc, in_maps, core_ids=list(range(NCORES)))
    return np.concatenate([res.results[c]["out"] for c in range(NCORES)], axis=0)
